# revision 40
# baseline (speedup 1.0000x reference)
"""BlanchotianAttention TRN2 kernel v3: 8 NeuronCores, data-parallel over
batch (2) x tensor-parallel over heads (4 heads/core).

Design (fp8-DoubleRow + ACT-strip softmax; cost-model driven):
  - All projections run as fp8e4m3 DoubleRow matmuls (0.5 cyc/row, 2x
    contraction per instruction) with a 3-term residual expansion
    (a8@b8 + ar8@b8 + a8@br8) keeping quantization error at bf16 level.
    Host ships x and 32*w as fp8 value+residual pairs laid out
    [p, m, pair, ...]: K=1024 = 4 DR matmuls per term.
  - Scores: q,k as 32-scaled fp8 [d, seq] tiles; per (head, jt) one DR
    matmul with K_p=64, a zeroed second pair on kt, and a stride-0 pair
    broadcast on qt: 256 cyc per [128j, 512i] block.
  - Softmax: 8 score strips ([3,2,2,2,2,2,2,2] j-tiles x 512 i) per
    (i-block, head) phase accumulate in PSUM with strictly alternating
    tags (even across phase boundaries, keeping the exp double-buffer
    tight); one ACT exp per strip (per-core per-head scale AP) writes
    bf16 p directly. PV = bf16 matmuls vs va=[32*v | ones]; the ones
    block doubles as the softmax denominator.
  - Void token: fake 17th j-tile riding the last strip; kt cols =
    [32*k_void, 0 x127], va rows zero except row 0 = [32*v_void | ones].
  - v-projection in per-(jt, head-pair) units so phase (0,h) only gates
    on its own head-pair: stage A spreads over phases 0-3 instead of
    flooding phases 0-1.
  - Normalize on DVE -> 32*out as fp8 (osb8) + fp8 residual (osbr8)
    feeding the 3-term fp8 out-projection; y partials leave as f32
    scaled by 1024; host divides and adds b_out. The last phase's norm
    is column-chunked and interleaved with its out-projection.
  - PSUM: strips 5 banks (sB 3 + sA 2), pvl 2 alternating, 1 transient
    bank. A drain queue releases PV only after its va-producing hooks
    are emitted (avoids PE queue deadlock); norms ride the drain.
"""
import sys

sys.path.insert(0, "/opt/trn_rl_repo")

import numpy as np

DIM, HEADS, B, N = 1024, 16, 2, 2048
D = DIM // HEADS          # 64
P = 128

_cache = {}
DEBUG = False

# strips per phase: (first j-tile, n j-tiles); j-tile 16 is the void tile
STRIPS = [(0, 3), (3, 2), (5, 2), (7, 2), (9, 2), (11, 2), (13, 2), (15, 2)]


def _build():
    import concourse.mybir as mybir
    import concourse.tile as tile
    from concourse import bacc

    F32 = mybir.dt.float32
    F8 = mybir.dt.float8e4
    BF16 = mybir.dt.bfloat16
    Exp = mybir.ActivationFunctionType.Exp
    DR = mybir.MatmulPerfMode.DoubleRow
    Mult = mybir.AluOpType.mult
    Sub = mybir.AluOpType.subtract

    nc = bacc.Bacc("TRN2", target_bir_lowering=False, debug=False)
    xT8 = nc.dram_tensor("xT8", [P, 4, 2, N], F8, kind="ExternalInput").ap()
    xr8 = nc.dram_tensor("xr8", [P, 4, 2, N], F8, kind="ExternalInput").ap()
    wq8 = nc.dram_tensor("wq8", [P, 4, 2, 768], F8, kind="ExternalInput").ap()
    wr8 = nc.dram_tensor("wr8", [P, 4, 2, 768], F8, kind="ExternalInput").ap()
    wo8 = nc.dram_tensor("wo8", [P, 2, DIM], F8, kind="ExternalInput").ap()
    wor8 = nc.dram_tensor("wor8", [P, 2, DIM], F8, kind="ExternalInput").ap()
    vk8 = nc.dram_tensor("vk8", [2, P, 1], F8, kind="ExternalInput").ap()
    vvo = nc.dram_tensor("vvo", [1, 4, P], BF16, kind="ExternalInput").ap()
    scal = nc.dram_tensor("scal", [P, 4], F32, kind="ExternalInput").ap()
    scha = nc.dram_tensor("scha", [P, 4], F32, kind="ExternalInput").ap()
    y = nc.dram_tensor("y", [N, DIM], F32, kind="ExternalOutput").ap()
    if DEBUG:
        dq = nc.dram_tensor("dq", [P, 2, N], F8, kind="ExternalOutput").ap()
        dk = nc.dram_tensor("dk", [P, 2, 2, 17 * P], F8,
                            kind="ExternalOutput").ap()
        dv = nc.dram_tensor("dv", [P, 17, 4, P], BF16,
                            kind="ExternalOutput").ap()
        do = nc.dram_tensor("do", [P, 2, 4, 512], F8,
                            kind="ExternalOutput").ap()
        dor = nc.dram_tensor("dor", [P, 2, 4, 512], F8,
                             kind="ExternalOutput").ap()

    with tile.TileContext(nc) as tc:
        with tc.tile_pool(name="persist", bufs=1) as pp, \
             tc.tile_pool(name="work", bufs=1) as wp, \
             tc.tile_pool(name="psum", bufs=1, space="PSUM") as ps:

            # ---- persistent SBUF ----
            x8 = pp.tile([P, 4, 2, N], F8)
            xr = pp.tile([P, 4, 2, N], F8)
            w8 = pp.tile([P, 4, 2, 768], F8)
            wr = pp.tile([P, 4, 2, 768], F8)
            wo = pp.tile([P, 2, DIM], F8)
            wor = pp.tile([P, 2, DIM], F8)
            qt8 = pp.tile([P, 2, N], F8)          # [.., t, i]: heads 2t,2t+1
            kt8 = pp.tile([P, 2, 2, 17 * P], F8)  # [.., t, pair, j(+void)]
            va = pp.tile([P, 17, 4, P], BF16)     # [j, jt, h, 32*v|ones]
            osb8 = pp.tile([P, 2, 4, 512], F8)    # [feat, pair, ic, i]
            osbr8 = pp.tile([P, 2, 4, 512], F8)
            sc_ap = pp.tile([P, 4], F32)
            sa_ap = pp.tile([P, 4], F32)
            wsrc = pp.tile([P, 512], BF16)

            # ---- DMA issues (priority order: phase-0's K/Q ladder is
            # DMA-gated, so x strips and their residuals go first) ----
            dma = nc.sync.dma_start
            dma(w8[:, :, :, 256:384], wq8[:, :, :, 256:384])   # K01
            dma(x8[:, :, :, 0:512], xT8[:, :, :, 0:512])
            dma(wr[:, :, :, 256:384], wr8[:, :, :, 256:384])
            dma(xr[:, :, :, 0:512], xr8[:, :, :, 0:512])
            dma(w8[:, :, :, 0:128], wq8[:, :, :, 0:128])       # Q01
            dma(wr[:, :, :, 0:128], wr8[:, :, :, 0:128])
            dma(sc_ap[:], scal)
            dma(sa_ap[:], scha)
            dma(x8[:, :, :, 512:1024], xT8[:, :, :, 512:1024])
            dma(xr[:, :, :, 512:1024], xr8[:, :, :, 512:1024])
            dma(w8[:, :, :, 512:768], wq8[:, :, :, 512:768])   # V
            dma(wr[:, :, :, 512:768], wr8[:, :, :, 512:768])
            dma(x8[:, :, :, 1024:1536], xT8[:, :, :, 1024:1536])
            dma(xr[:, :, :, 1024:1536], xr8[:, :, :, 1024:1536])
            dma(x8[:, :, :, 1536:2048], xT8[:, :, :, 1536:2048])
            dma(xr[:, :, :, 1536:2048], xr8[:, :, :, 1536:2048])
            dma(w8[:, :, :, 384:512], wq8[:, :, :, 384:512])   # K23
            dma(wr[:, :, :, 384:512], wr8[:, :, :, 384:512])
            dma(w8[:, :, :, 128:256], wq8[:, :, :, 128:256])   # Q23
            dma(wr[:, :, :, 128:256], wr8[:, :, :, 128:256])
            dma(wo[:], wo8)
            dma(wor[:], wor8)

            # ---- setup on Pool (memsets run under the DMA shadow) ----
            nc.gpsimd.memset(kt8[:, 0, :, :], 0.0)
            nc.gpsimd.memset(kt8[:, 1, :, :], 0.0)
            nc.gpsimd.memset(va[:], 0.0)
            nc.gpsimd.memset(va[:, 0:16, :, D:P], 1.0)
            nc.vector.memset(wsrc[:].bitcast(mybir.dt.uint16), 0)
            # void data lands AFTER the memsets (they'd zero it otherwise)
            dma(kt8[:, 0, 0, 16 * P:16 * P + 1], vk8[0])
            dma(kt8[:, 1, 0, 16 * P:16 * P + 1], vk8[1])
            dma(va[0:1, 16, :, :], vvo)
            # act-table preload off the critical path (dummy tiny exp)
            wdummy = pp.tile([1, 2], BF16)
            nc.scalar.activation(wdummy[:], wsrc[0:1, 0:2], Exp)

            # ---- stage A: q/k projections (fp8x3, DoubleRow) ----
            # ft: 0=Q01 1=Q23 2=K01 3=K23 (128 cols each); sc: seq block/512
            avdone = set()

            def emit_aqk(ft, sc, tag="trans"):
                acc = ps.tile([P, 512], F32, tag=tag, name=f"aqk_{ft}_{sc}")
                fs = slice(ft * P, (ft + 1) * P)
                ss = slice(sc * 512, (sc + 1) * 512)
                for term in range(3):
                    lhs_t, rhs_t = [(w8, x8), (wr, x8), (w8, xr)][term]
                    for m in range(4):
                        nc.tensor.matmul(
                            acc[:], lhs_t[:, m, :, fs], rhs_t[:, m, :, ss],
                            start=(term == 0 and m == 0),
                            stop=(term == 2 and m == 3), perf_mode=DR)
                if ft < 2:
                    nc.vector.tensor_copy(qt8[:, ft, ss], acc[:])
                else:
                    nc.vector.tensor_copy(kt8[:, ft - 2, 0, ss], acc[:])

            def aqk(ft, sc):
                return lambda: emit_aqk(ft, sc)

            # ---- stage A: v projection (fp8x3, DR, transposed) per
            # (j-tile, head-pair) so va availability is per head-pair ----
            def emit_av(jt, pr):
                acc = ps.tile([P, 512], F32, tag="trans",
                              name=f"av_{jt}_{pr}")
                js = slice(jt * P, (jt + 1) * P)
                ws = slice(512 + pr * P, 512 + (pr + 1) * P)
                for term in range(3):
                    lhs_t, rhs_t = [(x8, w8), (xr, w8), (x8, wr)][term]
                    for m in range(4):
                        nc.tensor.matmul(
                            acc[:, 0:P], lhs_t[:, m, :, js],
                            rhs_t[:, m, :, ws],
                            start=(term == 0 and m == 0),
                            stop=(term == 2 and m == 3), perf_mode=DR)
                nc.vector.tensor_copy(
                    va[:, jt, 2 * pr:2 * pr + 2, 0:D],
                    acc[:, 0:P].rearrange("p (h c) -> p h c", c=D))
                avdone.add((jt, pr))

            def av(jt, pr):
                return lambda: emit_av(jt, pr)

            # ---- scores: one DR matmul per (head, jt) block ----
            def emit_score(h, ic, jt, sblk):
                t, par = h // 2, (h % 2) * D
                lhsT = kt8[par:par + D, t, :, jt * P:(jt + 1) * P]
                rhs = qt8[par:par + D, t, ic * 512:(ic + 1) * 512]
                rhs = rhs[:, None, :].to_broadcast([D, 2, 512])
                nc.tensor.matmul(sblk, lhsT, rhs, start=True, stop=True,
                                 perf_mode=DR)

            # ---- PV for one strip ----
            def emit_pv(pvl, p, jt0, njt, h):
                for t in range(njt):
                    jt = jt0 + t
                    nc.tensor.matmul(pvl[:], va[:, jt, h, :], p[:, t, :],
                                     start=(jt == 0), stop=(jt == 16))

            # ---- normalize (DVE) -> osb8/osbr8 ----
            # Even heads write osb8 in place (partition-aligned); odd heads
            # stage through o8t and shift-copy (DVE mid-run, ACT in the
            # tail where ACT is idle).
            def emit_norm(ic, h, pvl, chunk=None, act_copy=False):
                t, par = h // 2, (h % 2) * D
                cs = slice(0, 512) if chunk is None else \
                    slice(chunk * P, (chunk + 1) * P)
                w = cs.stop - cs.start
                r_sb = wp.tile([D, 512], F32, tag="rsb", bufs=2,
                               name=f"rsb{ic}_{h}_{chunk}")
                nc.vector.reciprocal(r_sb[:, 0:w], pvl[D:P, cs])
                tmp = wp.tile([D, 512], F32, tag="ntmp", bufs=2,
                              name=f"ntmp{ic}_{h}_{chunk}")
                nc.vector.tensor_tensor(tmp[:, 0:w], pvl[0:D, cs],
                                        r_sb[:, 0:w], Mult)
                if par == 0:
                    o8 = osb8[0:D, t, ic, cs]
                    nc.vector.tensor_copy(o8, tmp[:, 0:w])
                    nc.vector.tensor_tensor(osbr8[0:D, t, ic, cs],
                                            tmp[:, 0:w], o8, Sub)
                else:
                    o8t = wp.tile([D, 512], F8, tag="o8t", bufs=2,
                                  name=f"o8t{ic}_{h}_{chunk}")
                    nc.vector.tensor_copy(o8t[:, 0:w], tmp[:, 0:w])
                    if act_copy:
                        nc.scalar.copy(osb8[par:par + D, t, ic, cs],
                                       o8t[:, 0:w])
                    else:
                        nc.vector.tensor_copy(osb8[par:par + D, t, ic, cs],
                                              o8t[:, 0:w])
                    nc.vector.tensor_tensor(osbr8[par:par + D, t, ic, cs],
                                            tmp[:, 0:w], o8t[:, 0:w], Sub)

            # ---- out projection (fp8x3, DR over K=256) ----
            def emit_oj(ic, it, oc, tag="trans", act_copy=False):
                yp = ps.tile([P, 512], F32, tag=tag, name=f"y{ic}_{it}_{oc}")
                its = slice(it * P, (it + 1) * P)
                ocs = slice(oc * 512, (oc + 1) * 512)
                for term in range(3):
                    lhs_t, rhs_t = [(osb8, wo), (osbr8, wo),
                                    (osb8, wor)][term]
                    nc.tensor.matmul(
                        yp[:], lhs_t[:, :, ic, its], rhs_t[:, :, ocs],
                        start=(term == 0), stop=(term == 2), perf_mode=DR)
                ysb = wp.tile([P, 512], F32, tag="ysb", bufs=6,
                              name=f"ysb{ic}_{it}_{oc}")
                if act_copy:
                    nc.scalar.copy(ysb[:], yp[:])
                else:
                    nc.vector.tensor_copy(ysb[:], yp[:])
                dma(y[ic * 512 + it * P:ic * 512 + (it + 1) * P, ocs],
                    ysb[:])

            # mid-phase outproj: alternate the PSUM bank between the trans
            # bank and the idle (previous phase's) pvl bank, and alternate
            # the PSUM->SBUF copy between DVE and ACT, so consecutive oj
            # units don't serialize on one bank's WAR chain.
            ojst = {"n": 0}

            def oj(ic, it, oc):
                def f():
                    n = ojst["n"]
                    ojst["n"] += 1
                    tag = "trans" if n % 2 == 0 else \
                        f"pvl{1 - cur_pi[0] % 2}"
                    emit_oj(ic, it, oc, tag=tag, act_copy=(n % 2 == 1))
                return f

            cur_pi = [0]

            # ---- hook schedule: (phase, strip-slot) -> [thunks] ----
            # hooks[0] goes before the slot's PV drain, the rest after.
            # Phases are (h, ic): p = 4h + ic. kt/qt/va ladders for head
            # pair 1 spread over phases 4-7; outproj rides phases 13-15.
            hooks = {
                (0, 0): [aqk(2, 1)], (0, 1): [aqk(2, 2)],
                (0, 2): [av(0, 0)], (0, 3): [aqk(2, 3)],
                (0, 4): [av(1, 0), av(2, 0)], (0, 5): [aqk(0, 1)],
                (0, 6): [av(3, 0), av(4, 0)], (0, 7): [av(5, 0), av(6, 0)],
                (1, 0): [av(7, 0)], (1, 1): [av(8, 0), av(9, 0)],
                (1, 2): [av(10, 0), av(11, 0)], (1, 3): [aqk(0, 2)],
                (1, 4): [av(12, 0), av(13, 0)],
                (1, 5): [av(14, 0), av(15, 0)], (1, 6): [aqk(3, 0)],
                (2, 1): [aqk(3, 1)], (2, 3): [aqk(0, 3)],
                (2, 5): [aqk(3, 2)],
                (3, 1): [aqk(3, 3)], (3, 3): [aqk(1, 0)],
                (4, 1): [aqk(1, 1)],
                (5, 1): [aqk(1, 2)], (5, 3): [av(0, 1)],
                (5, 4): [av(1, 1), av(2, 1)], (5, 6): [av(3, 1), av(4, 1)],
                (6, 0): [av(5, 1)], (6, 1): [av(6, 1), av(7, 1)],
                (6, 3): [av(8, 1), av(9, 1)], (6, 5): [av(10, 1), av(11, 1)],
                (6, 6): [aqk(1, 3)],
                (7, 0): [av(12, 1)], (7, 1): [av(13, 1), av(14, 1)],
                (7, 3): [av(15, 1)],
                # outproj: oj(ic) after norm(h3, ic) = phase 12+ic
                (13, 1): [oj(0, 0, 0)], (13, 2): [oj(0, 0, 1)],
                (13, 3): [oj(0, 1, 0)], (13, 4): [oj(0, 1, 1)],
                (13, 5): [oj(0, 2, 0)], (13, 6): [oj(0, 2, 1)],
                (13, 7): [oj(0, 3, 0)],
                (14, 1): [oj(1, 0, 0), oj(0, 3, 1)],
                (14, 2): [oj(1, 0, 1)],
                (14, 3): [oj(1, 1, 0)], (14, 4): [oj(1, 1, 1)],
                (14, 5): [oj(1, 2, 0)], (14, 6): [oj(1, 2, 1)],
                (14, 7): [oj(1, 3, 0)],
                (15, 1): [oj(2, 0, 0), oj(1, 3, 1)],
                (15, 2): [oj(2, 0, 1)],
                (15, 3): [oj(2, 1, 0)], (15, 4): [oj(2, 1, 1)],
                (15, 5): [oj(2, 2, 0)], (15, 6): [oj(2, 2, 1)],
                (15, 7): [oj(2, 3, 0), oj(2, 3, 1)],
            }

            # ---- warmup: keep PE ramping while DMA lands ----
            wtags = ["trans", "pvl0", "pvl1"]
            for w in range(9):
                wacc = ps.tile([P, 512], F32, tag=wtags[w % 3],
                               name=f"warm_{w}")
                nc.tensor.matmul(wacc[:], wsrc[:, 0:128], wsrc[:],
                                 start=True, stop=True)

            # ---- prologue stage A: K01 sc0 + Q01 sc0 ----
            emit_aqk(2, 0, tag="trans")
            emit_aqk(0, 0, tag="pvl0")

            # ---- PV drain queue ----
            from collections import deque
            pvq = deque()  # (slot, pi, ic, h, islast, pvl, p, jt0, njt)

            def covered(jt0, njt, h):
                return all(jt == 16 or (jt, h // 2) in avdone
                           for jt in range(jt0, jt0 + njt))

            def drain(cur_slot, force_pi=-1):
                while pvq:
                    (slot, qpi, qic, qh, islast, pvl, p, jt0, njt) = pvq[0]
                    force = qpi <= force_pi
                    if not force and not (slot < cur_slot
                                          and covered(jt0, njt, qh)):
                        break
                    if force:
                        assert covered(jt0, njt, qh), (
                            f"forced PV before va ready: phase {qpi} "
                            f"jts {jt0}..{jt0 + njt - 1}")
                    pvq.popleft()
                    emit_pv(pvl, p, jt0, njt, qh)
                    if islast and qpi < 15:
                        emit_norm(qic, qh, pvl)

            # ---- main loop ----
            phases = [(ic, h) for h in range(4) for ic in range(4)]
            slot = 0
            last_pvl = None
            for pi, (ic, h) in enumerate(phases):
                cur_pi[0] = pi
                drain(slot, force_pi=pi - 2)
                pvl = ps.tile([P, 512], F32, tag=f"pvl{pi % 2}",
                              name=f"pvl_{pi}")
                if pi == 15:
                    last_pvl = pvl
                for si, (jt0, njt) in enumerate(STRIPS):
                    stag = "sB" if si % 2 == 0 else "sA"
                    s = ps.tile([P, 3 if si % 2 == 0 else 2, 512], F32,
                                tag=stag, name=f"s_{pi}_{si}")
                    for t in range(njt):
                        emit_score(h, ic, jt0 + t, s[:, t, :])
                    p = wp.tile([P, 3 if si % 2 == 0 else 2, 512], BF16,
                                tag=f"p{si % 2}", bufs=8,
                                name=f"p_{pi}_{si}")
                    if si >= len(STRIPS) - 2:
                        # DVE Schraudolph exp: bf16 bits = floor(z*A + B).
                        # This strip is jt15 + the fake void tile, so only
                        # ~6% of real softmax weight sees the approximation.
                        nc.vector.tensor_scalar(
                            p[:, 0:njt, :].bitcast(mybir.dt.uint16),
                            s[:, 0:njt, :], sa_ap[:, h:h + 1], 16249.1,
                            Mult, mybir.AluOpType.add)
                    else:
                        nc.scalar.activation(p[:, 0:njt, :], s[:, 0:njt, :],
                                             Exp, scale=sc_ap[:, h:h + 1])
                    hk = hooks.get((pi, si), [])
                    if hk and si > 0:
                        hk[0]()
                    drain(slot)
                    if hk and si == 0:
                        hk[0]()
                    for f in hk[1:]:
                        f()
                    drain(slot)
                    pvq.append((slot, pi, ic, h, si == len(STRIPS) - 1,
                                pvl, p, jt0, njt))
                    slot += 1

            # ---- tail: drain all; chunked last norm + outproj(3) ----
            drain(slot, force_pi=15)
            ytags = ["trans", "pvl0", "pvl1"]
            for chunk in range(4):
                emit_norm(3, 3, last_pvl, chunk=chunk, act_copy=True)
                emit_oj(3, chunk, 0, tag=ytags[(2 * chunk) % 3],
                        act_copy=(chunk % 2 == 1))
                emit_oj(3, chunk, 1, tag=ytags[(2 * chunk + 1) % 3],
                        act_copy=(chunk % 2 == 0))
            if DEBUG:
                dma(dq, qt8[:])
                dma(dk, kt8[:])
                dma(dv, va[:])
                dma(do, osb8[:])
                dma(dor, osbr8[:])

    nc.compile()
    return nc


def _prep_inputs(x, w_qkv, w_out, b_out, void_q, void_k, void_v,
                 attention_trace, temperature_factor):
    """Host-side sharding / fp8 layout prep. Returns in_maps for 8 cores."""
    import ml_dtypes
    F8 = ml_dtypes.float8_e4m3
    BF = ml_dtypes.bfloat16

    temp = np.maximum(1.0 + np.abs(attention_trace) * temperature_factor,
                      1.0).reshape(HEADS).astype(np.float32)
    scale_g = ((DIM ** -0.5) / 1024.0 / temp).astype(np.float32)  # [16]
    scha_g = (184.6649652337873 * scale_g).astype(np.float32)

    def split8(a):
        a8 = np.asarray(a, F8)
        ar = np.asarray(a - a8.astype(np.float32), F8)
        return a8, ar

    def pack_k(a):  # [1024, C] -> [128, 4, 2, C]
        return np.ascontiguousarray(
            a.reshape(4, 2, P, a.shape[1]).transpose(2, 0, 1, 3))

    w32 = (32.0 * w_qkv).astype(np.float32)
    w8f, wrf = split8(w32)
    wo32 = (32.0 * w_out).astype(np.float32)
    wo8f, worf = split8(wo32)
    vk = (32.0 * void_k.reshape(HEADS, D)).astype(np.float32)
    vv = (32.0 * void_v.reshape(HEADS, D)).astype(np.float32)

    xt8, xtr = [], []
    for b in range(B):
        xt = np.ascontiguousarray(x[b].T).astype(np.float32)
        a8, ar = split8(xt)
        xt8.append(pack_k(a8))
        xtr.append(pack_k(ar))

    in_maps = []
    for core in range(8):
        b, hg = divmod(core, 4)
        h0 = hg * 4
        cs = slice(h0 * D, h0 * D + 256)
        cols = np.concatenate([
            np.arange(h0 * D, h0 * D + 256),
            np.arange(DIM + h0 * D, DIM + h0 * D + 256),
            np.arange(2 * DIM + h0 * D, 2 * DIM + h0 * D + 256)])
        vvo = np.ones((1, 4, P), np.float32)
        vvo[0, :, 0:D] = vv[h0:h0 + 4]
        in_maps.append({
            "xT8": xt8[b], "xr8": xtr[b],
            "wq8": pack_k(w8f[:, cols]), "wr8": pack_k(wrf[:, cols]),
            "wo8": np.ascontiguousarray(
                wo8f[cs].reshape(2, P, DIM).transpose(1, 0, 2)),
            "wor8": np.ascontiguousarray(
                worf[cs].reshape(2, P, DIM).transpose(1, 0, 2)),
            "vk8": np.ascontiguousarray(
                vk[h0:h0 + 4].reshape(2, P, 1)).astype(F8),
            "vvo": vvo.astype(BF),
            "scal": np.tile(scale_g[h0:h0 + 4], (P, 1)),
            "scha": np.tile(scha_g[h0:h0 + 4], (P, 1)),
        })
    return in_maps


def _run(in_maps, trace=False):
    from concourse import bass_utils
    if "nc" not in _cache:
        _cache["nc"] = _build()
    return bass_utils.run_bass_kernel_spmd(
        _cache["nc"], in_maps, core_ids=list(range(8)), trace=trace)


def kernel(x, w_qkv, w_out, b_out, void_q, void_k, void_v,
           attention_trace, temperature_factor):
    args = [np.asarray(a, dtype=np.float32) for a in
            (x, w_qkv, w_out, b_out, void_q, void_k, void_v,
             attention_trace, temperature_factor)]
    in_maps = _prep_inputs(*args)
    res = _run(in_maps)
    out = np.zeros((B, N, DIM), np.float32)
    for core in range(8):
        b = core // 4
        out[b] += np.asarray(res.results[core]["y"], dtype=np.float32)
    out *= (1.0 / 1024.0)
    out += args[3][None, None, :]                      # b_out
    return out


# revision 41
# speedup vs baseline: 1.0050x; 1.0050x over previous
"""BlanchotianAttention TRN2 kernel v3: 8 NeuronCores, data-parallel over
batch (2) x tensor-parallel over heads (4 heads/core).

Design (fp8-DoubleRow + ACT-strip softmax; cost-model driven):
  - All projections run as fp8e4m3 DoubleRow matmuls (0.5 cyc/row, 2x
    contraction per instruction) with a 3-term residual expansion
    (a8@b8 + ar8@b8 + a8@br8) keeping quantization error at bf16 level.
    Host ships x and 32*w as fp8 value+residual pairs laid out
    [p, m, pair, ...]: K=1024 = 4 DR matmuls per term.
  - Scores: q,k as 32-scaled fp8 [d, seq] tiles; per (head, jt) one DR
    matmul with K_p=64, a zeroed second pair on kt, and a stride-0 pair
    broadcast on qt: 256 cyc per [128j, 512i] block.
  - Softmax: 8 score strips ([3,2,2,2,2,2,2,2] j-tiles x 512 i) per
    (i-block, head) phase accumulate in PSUM with strictly alternating
    tags (even across phase boundaries, keeping the exp double-buffer
    tight); one ACT exp per strip (per-core per-head scale AP) writes
    bf16 p directly. PV = bf16 matmuls vs va=[32*v | ones]; the ones
    block doubles as the softmax denominator.
  - Void token: fake 17th j-tile riding the last strip; kt cols =
    [32*k_void, 0 x127], va rows zero except row 0 = [32*v_void | ones].
  - v-projection in per-(jt, head-pair) units so phase (0,h) only gates
    on its own head-pair: stage A spreads over phases 0-3 instead of
    flooding phases 0-1.
  - Normalize on DVE -> 32*out as fp8 (osb8) + fp8 residual (osbr8)
    feeding the 3-term fp8 out-projection; y partials leave as f32
    scaled by 1024; host divides and adds b_out. The last phase's norm
    is column-chunked and interleaved with its out-projection.
  - PSUM: strips 5 banks (sB 3 + sA 2), pvl 2 alternating, 1 transient
    bank. A drain queue releases PV only after its va-producing hooks
    are emitted (avoids PE queue deadlock); norms ride the drain.
"""
import sys

sys.path.insert(0, "/opt/trn_rl_repo")

import numpy as np

DIM, HEADS, B, N = 1024, 16, 2, 2048
D = DIM // HEADS          # 64
P = 128

_cache = {}
DEBUG = False

# strips per phase: (first j-tile, n j-tiles); j-tile 16 is the void tile
STRIPS = [(0, 3), (3, 2), (5, 2), (7, 2), (9, 2), (11, 2), (13, 2), (15, 2)]


def _build():
    import concourse.mybir as mybir
    import concourse.tile as tile
    from concourse import bacc

    F32 = mybir.dt.float32
    F8 = mybir.dt.float8e4
    BF16 = mybir.dt.bfloat16
    Exp = mybir.ActivationFunctionType.Exp
    DR = mybir.MatmulPerfMode.DoubleRow
    Mult = mybir.AluOpType.mult
    Sub = mybir.AluOpType.subtract

    nc = bacc.Bacc("TRN2", target_bir_lowering=False, debug=False)
    xT8 = nc.dram_tensor("xT8", [P, 4, 2, N], F8, kind="ExternalInput").ap()
    xr8 = nc.dram_tensor("xr8", [P, 4, 2, N], F8, kind="ExternalInput").ap()
    wq8 = nc.dram_tensor("wq8", [P, 4, 2, 768], F8, kind="ExternalInput").ap()
    wr8 = nc.dram_tensor("wr8", [P, 4, 2, 768], F8, kind="ExternalInput").ap()
    wo8 = nc.dram_tensor("wo8", [P, 2, DIM], F8, kind="ExternalInput").ap()
    wor8 = nc.dram_tensor("wor8", [P, 2, DIM], F8, kind="ExternalInput").ap()
    vk8 = nc.dram_tensor("vk8", [2, P, 1], F8, kind="ExternalInput").ap()
    vvo = nc.dram_tensor("vvo", [1, 4, P], BF16, kind="ExternalInput").ap()
    scal = nc.dram_tensor("scal", [P, 4], F32, kind="ExternalInput").ap()
    scha = nc.dram_tensor("scha", [P, 4], F32, kind="ExternalInput").ap()
    y = nc.dram_tensor("y", [N, DIM], F32, kind="ExternalOutput").ap()
    if DEBUG:
        dq = nc.dram_tensor("dq", [P, 2, N], F8, kind="ExternalOutput").ap()
        dk = nc.dram_tensor("dk", [P, 2, 2, 17 * P], F8,
                            kind="ExternalOutput").ap()
        dv = nc.dram_tensor("dv", [P, 17, 4, P], BF16,
                            kind="ExternalOutput").ap()
        do = nc.dram_tensor("do", [P, 2, 4, 512], F8,
                            kind="ExternalOutput").ap()
        dor = nc.dram_tensor("dor", [P, 2, 4, 512], F8,
                             kind="ExternalOutput").ap()

    with tile.TileContext(nc) as tc:
        with tc.tile_pool(name="persist", bufs=1) as pp, \
             tc.tile_pool(name="work", bufs=1) as wp, \
             tc.tile_pool(name="psum", bufs=1, space="PSUM") as ps:

            # ---- persistent SBUF ----
            x8 = pp.tile([P, 4, 2, N], F8)
            xr = pp.tile([P, 4, 2, N], F8)
            w8 = pp.tile([P, 4, 2, 768], F8)
            wr = pp.tile([P, 4, 2, 768], F8)
            wo = pp.tile([P, 2, DIM], F8)
            wor = pp.tile([P, 2, DIM], F8)
            qt8 = pp.tile([P, 2, N], F8)          # [.., t, i]: heads 2t,2t+1
            kt8 = pp.tile([P, 2, 2, 17 * P], F8)  # [.., t, pair, j(+void)]
            va = pp.tile([P, 17, 4, P], BF16)     # [j, jt, h, 32*v|ones]
            osb8 = pp.tile([P, 2, 4, 512], F8)    # [feat, pair, ic, i]
            osbr8 = pp.tile([P, 2, 4, 512], F8)
            sc_ap = pp.tile([P, 4], F32)
            sa_ap = pp.tile([P, 4], F32)
            wsrc = pp.tile([P, 512], BF16)

            # ---- DMA issues (priority order: phase-0's K/Q ladder is
            # DMA-gated, so x strips and their residuals go first) ----
            dma = nc.sync.dma_start
            dma(w8[:, :, :, 256:384], wq8[:, :, :, 256:384])   # K01
            dma(x8[:, :, :, 0:512], xT8[:, :, :, 0:512])
            dma(wr[:, :, :, 256:384], wr8[:, :, :, 256:384])
            dma(xr[:, :, :, 0:512], xr8[:, :, :, 0:512])
            dma(w8[:, :, :, 0:128], wq8[:, :, :, 0:128])       # Q01
            dma(wr[:, :, :, 0:128], wr8[:, :, :, 0:128])
            dma(sc_ap[:], scal)
            dma(sa_ap[:], scha)
            dma(x8[:, :, :, 512:1024], xT8[:, :, :, 512:1024])
            dma(xr[:, :, :, 512:1024], xr8[:, :, :, 512:1024])
            dma(w8[:, :, :, 512:768], wq8[:, :, :, 512:768])   # V
            dma(wr[:, :, :, 512:768], wr8[:, :, :, 512:768])
            dma(x8[:, :, :, 1024:1536], xT8[:, :, :, 1024:1536])
            dma(xr[:, :, :, 1024:1536], xr8[:, :, :, 1024:1536])
            dma(x8[:, :, :, 1536:2048], xT8[:, :, :, 1536:2048])
            dma(xr[:, :, :, 1536:2048], xr8[:, :, :, 1536:2048])
            dma(w8[:, :, :, 384:512], wq8[:, :, :, 384:512])   # K23
            dma(wr[:, :, :, 384:512], wr8[:, :, :, 384:512])
            dma(w8[:, :, :, 128:256], wq8[:, :, :, 128:256])   # Q23
            dma(wr[:, :, :, 128:256], wr8[:, :, :, 128:256])
            dma(wo[:], wo8)
            dma(wor[:], wor8)

            # ---- setup on Pool (memsets run under the DMA shadow) ----
            nc.gpsimd.memset(kt8[:, 0, :, :], 0.0)
            nc.gpsimd.memset(kt8[:, 1, :, :], 0.0)
            nc.gpsimd.memset(va[:], 0.0)
            nc.gpsimd.memset(va[:, 0:16, :, D:P], 1.0)
            nc.vector.memset(wsrc[:].bitcast(mybir.dt.uint16), 0)
            # void data lands AFTER the memsets (they'd zero it otherwise)
            dma(kt8[:, 0, 0, 16 * P:16 * P + 1], vk8[0])
            dma(kt8[:, 1, 0, 16 * P:16 * P + 1], vk8[1])
            dma(va[0:1, 16, :, :], vvo)
            # act-table preload off the critical path (dummy tiny exp)
            wdummy = pp.tile([1, 2], BF16)
            nc.scalar.activation(wdummy[:], wsrc[0:1, 0:2], Exp)

            # ---- stage A: q/k projections (fp8x3, DoubleRow) ----
            # ft: 0=Q01 1=Q23 2=K01 3=K23 (128 cols each); sc: seq block/512
            avdone = set()

            def emit_aqk(ft, sc, tag="trans"):
                acc = ps.tile([P, 512], F32, tag=tag, name=f"aqk_{ft}_{sc}")
                fs = slice(ft * P, (ft + 1) * P)
                ss = slice(sc * 512, (sc + 1) * 512)
                for term in range(3):
                    lhs_t, rhs_t = [(w8, x8), (wr, x8), (w8, xr)][term]
                    for m in range(4):
                        nc.tensor.matmul(
                            acc[:], lhs_t[:, m, :, fs], rhs_t[:, m, :, ss],
                            start=(term == 0 and m == 0),
                            stop=(term == 2 and m == 3), perf_mode=DR)
                if ft < 2:
                    nc.vector.tensor_copy(qt8[:, ft, ss], acc[:])
                else:
                    nc.vector.tensor_copy(kt8[:, ft - 2, 0, ss], acc[:])

            def aqk(ft, sc):
                return lambda: emit_aqk(ft, sc)

            # ---- stage A: v projection (fp8x3, DR, transposed) per
            # (j-tile, head-pair) so va availability is per head-pair ----
            def emit_av(jt, pr):
                acc = ps.tile([P, 512], F32, tag="trans",
                              name=f"av_{jt}_{pr}")
                js = slice(jt * P, (jt + 1) * P)
                ws = slice(512 + pr * P, 512 + (pr + 1) * P)
                for term in range(3):
                    lhs_t, rhs_t = [(x8, w8), (xr, w8), (x8, wr)][term]
                    for m in range(4):
                        nc.tensor.matmul(
                            acc[:, 0:P], lhs_t[:, m, :, js],
                            rhs_t[:, m, :, ws],
                            start=(term == 0 and m == 0),
                            stop=(term == 2 and m == 3), perf_mode=DR)
                nc.vector.tensor_copy(
                    va[:, jt, 2 * pr:2 * pr + 2, 0:D],
                    acc[:, 0:P].rearrange("p (h c) -> p h c", c=D))
                avdone.add((jt, pr))

            def av(jt, pr):
                return lambda: emit_av(jt, pr)

            # ---- scores: one DR matmul per (head, jt) block ----
            def emit_score(h, ic, jt, sblk):
                t, par = h // 2, (h % 2) * D
                lhsT = kt8[par:par + D, t, :, jt * P:(jt + 1) * P]
                rhs = qt8[par:par + D, t, ic * 512:(ic + 1) * 512]
                rhs = rhs[:, None, :].to_broadcast([D, 2, 512])
                nc.tensor.matmul(sblk, lhsT, rhs, start=True, stop=True,
                                 perf_mode=DR)

            # ---- PV for one strip ----
            def emit_pv(pvl, p, jt0, njt, h):
                for t in range(njt):
                    jt = jt0 + t
                    nc.tensor.matmul(pvl[:], va[:, jt, h, :], p[:, t, :],
                                     start=(jt == 0), stop=(jt == 16))

            # ---- normalize (DVE) -> osb8/osbr8 ----
            # Even heads write osb8 in place (partition-aligned); odd heads
            # stage through o8t and shift-copy (DVE mid-run, ACT in the
            # tail where ACT is idle).
            def emit_norm(ic, h, pvl, chunk=None, act_copy=False):
                t, par = h // 2, (h % 2) * D
                cs = slice(0, 512) if chunk is None else \
                    slice(chunk * P, (chunk + 1) * P)
                w = cs.stop - cs.start
                r_sb = wp.tile([D, 512], F32, tag="rsb", bufs=2,
                               name=f"rsb{ic}_{h}_{chunk}")
                nc.vector.reciprocal(r_sb[:, 0:w], pvl[D:P, cs])
                tmp = wp.tile([D, 512], F32, tag="ntmp", bufs=2,
                              name=f"ntmp{ic}_{h}_{chunk}")
                nc.vector.tensor_tensor(tmp[:, 0:w], pvl[0:D, cs],
                                        r_sb[:, 0:w], Mult)
                if par == 0:
                    o8 = osb8[0:D, t, ic, cs]
                    nc.vector.tensor_copy(o8, tmp[:, 0:w])
                    nc.vector.tensor_tensor(osbr8[0:D, t, ic, cs],
                                            tmp[:, 0:w], o8, Sub)
                else:
                    o8t = wp.tile([D, 512], F8, tag="o8t", bufs=2,
                                  name=f"o8t{ic}_{h}_{chunk}")
                    nc.vector.tensor_copy(o8t[:, 0:w], tmp[:, 0:w])
                    if act_copy:
                        nc.scalar.copy(osb8[par:par + D, t, ic, cs],
                                       o8t[:, 0:w])
                    else:
                        nc.vector.tensor_copy(osb8[par:par + D, t, ic, cs],
                                              o8t[:, 0:w])
                    nc.vector.tensor_tensor(osbr8[par:par + D, t, ic, cs],
                                            tmp[:, 0:w], o8t[:, 0:w], Sub)

            # ---- out projection (fp8x3, DR over K=256) ----
            def emit_oj(ic, it, oc, tag="trans", act_copy=False):
                yp = ps.tile([P, 512], F32, tag=tag, name=f"y{ic}_{it}_{oc}")
                its = slice(it * P, (it + 1) * P)
                ocs = slice(oc * 512, (oc + 1) * 512)
                for term in range(3):
                    lhs_t, rhs_t = [(osb8, wo), (osbr8, wo),
                                    (osb8, wor)][term]
                    nc.tensor.matmul(
                        yp[:], lhs_t[:, :, ic, its], rhs_t[:, :, ocs],
                        start=(term == 0), stop=(term == 2), perf_mode=DR)
                ysb = wp.tile([P, 512], F32, tag="ysb", bufs=6,
                              name=f"ysb{ic}_{it}_{oc}")
                if act_copy:
                    nc.scalar.copy(ysb[:], yp[:])
                else:
                    nc.vector.tensor_copy(ysb[:], yp[:])
                dma(y[ic * 512 + it * P:ic * 512 + (it + 1) * P, ocs],
                    ysb[:])

            # mid-phase outproj: alternate the PSUM bank between the trans
            # bank and the idle (previous phase's) pvl bank, and alternate
            # the PSUM->SBUF copy between DVE and ACT, so consecutive oj
            # units don't serialize on one bank's WAR chain.
            ojst = {"n": 0}

            def oj(ic, it, oc):
                def f():
                    n = ojst["n"]
                    ojst["n"] += 1
                    tag = "trans" if n % 2 == 0 else \
                        f"pvl{1 - cur_pi[0] % 2}"
                    emit_oj(ic, it, oc, tag=tag, act_copy=(n % 2 == 1))
                return f

            cur_pi = [0]

            # ---- hook schedule: (phase, strip-slot) -> [thunks] ----
            # hooks[0] goes before the slot's PV drain, the rest after.
            # Phases are (h, ic): p = 4h + ic. kt/qt/va ladders for head
            # pair 1 spread over phases 4-7; outproj rides phases 13-15.
            hooks = {
                (0, 0): [aqk(2, 1)], (0, 1): [aqk(2, 2)],
                (0, 2): [av(0, 0)], (0, 3): [aqk(2, 3)],
                (0, 4): [av(1, 0), av(2, 0)], (0, 5): [aqk(0, 1)],
                (0, 6): [av(3, 0), av(4, 0)], (0, 7): [av(5, 0), av(6, 0)],
                (1, 0): [av(7, 0)], (1, 1): [av(8, 0), av(9, 0)],
                (1, 2): [av(10, 0), av(11, 0)], (1, 3): [aqk(0, 2)],
                (1, 4): [av(12, 0), av(13, 0)],
                (1, 5): [av(14, 0), av(15, 0)], (1, 6): [aqk(3, 0)],
                (2, 1): [aqk(3, 1)], (2, 3): [aqk(0, 3)],
                (2, 5): [aqk(3, 2)],
                (3, 1): [aqk(3, 3)], (3, 3): [aqk(1, 0)],
                (4, 1): [aqk(1, 1)],
                (5, 1): [aqk(1, 2)], (5, 3): [av(0, 1)],
                (5, 4): [av(1, 1), av(2, 1)], (5, 6): [av(3, 1), av(4, 1)],
                (6, 0): [av(5, 1)], (6, 1): [av(6, 1), av(7, 1)],
                (6, 3): [av(8, 1), av(9, 1)], (6, 5): [av(10, 1), av(11, 1)],
                (6, 6): [aqk(1, 3)],
                (7, 0): [av(12, 1)], (7, 1): [av(13, 1), av(14, 1)],
                (7, 3): [av(15, 1)],
                # outproj: oj(ic) after norm(h3, ic) = phase 12+ic
                (13, 1): [oj(0, 0, 0)], (13, 2): [oj(0, 0, 1)],
                (13, 3): [oj(0, 1, 0)], (13, 4): [oj(0, 1, 1)],
                (13, 5): [oj(0, 2, 0)], (13, 6): [oj(0, 2, 1)],
                (13, 7): [oj(0, 3, 0)],
                (14, 1): [oj(1, 0, 0), oj(0, 3, 1)],
                (14, 2): [oj(1, 0, 1)],
                (14, 3): [oj(1, 1, 0)], (14, 4): [oj(1, 1, 1)],
                (14, 5): [oj(1, 2, 0)], (14, 6): [oj(1, 2, 1)],
                (14, 7): [oj(1, 3, 0)],
                (15, 1): [oj(2, 0, 0), oj(1, 3, 1)],
                (15, 2): [oj(2, 0, 1)],
                (15, 3): [oj(2, 1, 0)], (15, 4): [oj(2, 1, 1)],
                (15, 5): [oj(2, 2, 0)], (15, 6): [oj(2, 2, 1)],
                (15, 7): [oj(2, 3, 0), oj(2, 3, 1)],
            }

            # ---- warmup: keep PE ramping while DMA lands ----
            wtags = ["trans", "pvl0", "pvl1"]
            for w in range(9):
                wacc = ps.tile([P, 512], F32, tag=wtags[w % 3],
                               name=f"warm_{w}")
                nc.tensor.matmul(wacc[:], wsrc[:, 0:128], wsrc[:],
                                 start=True, stop=True)

            # ---- prologue stage A: K01 sc0 + Q01 sc0 ----
            emit_aqk(2, 0, tag="trans")
            emit_aqk(0, 0, tag="pvl0")

            # ---- PV drain queue ----
            from collections import deque
            pvq = deque()  # (slot, pi, ic, h, islast, pvl, p, jt0, njt)

            def covered(jt0, njt, h):
                return all(jt == 16 or (jt, h // 2) in avdone
                           for jt in range(jt0, jt0 + njt))

            def drain(cur_slot, force_pi=-1):
                while pvq:
                    (slot, qpi, qic, qh, islast, pvl, p, jt0, njt) = pvq[0]
                    force = qpi <= force_pi
                    if not force and not (slot < cur_slot
                                          and covered(jt0, njt, qh)):
                        break
                    if force:
                        assert covered(jt0, njt, qh), (
                            f"forced PV before va ready: phase {qpi} "
                            f"jts {jt0}..{jt0 + njt - 1}")
                    pvq.popleft()
                    emit_pv(pvl, p, jt0, njt, qh)
                    if islast and qpi < 15:
                        emit_norm(qic, qh, pvl)

            # ---- main loop ----
            phases = [(ic, h) for h in range(4) for ic in range(4)]
            slot = 0
            last_pvl = None
            for pi, (ic, h) in enumerate(phases):
                cur_pi[0] = pi
                drain(slot, force_pi=pi - 2)
                pvl = ps.tile([P, 512], F32, tag=f"pvl{pi % 2}",
                              name=f"pvl_{pi}")
                if pi == 15:
                    last_pvl = pvl
                for si, (jt0, njt) in enumerate(STRIPS):
                    stag = "sB" if si % 2 == 0 else "sA"
                    s = ps.tile([P, 3 if si % 2 == 0 else 2, 512], F32,
                                tag=stag, name=f"s_{pi}_{si}")
                    for t in range(njt):
                        emit_score(h, ic, jt0 + t, s[:, t, :])
                    p = wp.tile([P, 3 if si % 2 == 0 else 2, 512], BF16,
                                tag=f"p{si % 2}", bufs=8,
                                name=f"p_{pi}_{si}")
                    if si == len(STRIPS) - 1:
                        # DVE Schraudolph exp: bf16 bits = floor(z*A + B).
                        # This strip is jt15 + the fake void tile, so only
                        # ~6% of real softmax weight sees the approximation.
                        nc.vector.tensor_scalar(
                            p[:, 0:njt, :].bitcast(mybir.dt.uint16),
                            s[:, 0:njt, :], sa_ap[:, h:h + 1], 16249.1,
                            Mult, mybir.AluOpType.add)
                    else:
                        nc.scalar.activation(p[:, 0:njt, :], s[:, 0:njt, :],
                                             Exp, scale=sc_ap[:, h:h + 1])
                    hk = hooks.get((pi, si), [])
                    if hk and si > 0:
                        hk[0]()
                    drain(slot)
                    if hk and si == 0:
                        hk[0]()
                    for f in hk[1:]:
                        f()
                    drain(slot)
                    pvq.append((slot, pi, ic, h, si == len(STRIPS) - 1,
                                pvl, p, jt0, njt))
                    slot += 1

            # ---- tail: drain all; chunked last norm + outproj(3) ----
            drain(slot, force_pi=15)
            ytags = ["trans", "pvl0", "pvl1"]
            for chunk in range(4):
                emit_norm(3, 3, last_pvl, chunk=chunk, act_copy=True)
                emit_oj(3, chunk, 0, tag=ytags[(2 * chunk) % 3],
                        act_copy=(chunk % 2 == 1))
                emit_oj(3, chunk, 1, tag=ytags[(2 * chunk + 1) % 3],
                        act_copy=(chunk % 2 == 0))
            if DEBUG:
                dma(dq, qt8[:])
                dma(dk, kt8[:])
                dma(dv, va[:])
                dma(do, osb8[:])
                dma(dor, osbr8[:])

    nc.compile()
    return nc


def _prep_inputs(x, w_qkv, w_out, b_out, void_q, void_k, void_v,
                 attention_trace, temperature_factor):
    """Host-side sharding / fp8 layout prep. Returns in_maps for 8 cores."""
    import ml_dtypes
    F8 = ml_dtypes.float8_e4m3
    BF = ml_dtypes.bfloat16

    temp = np.maximum(1.0 + np.abs(attention_trace) * temperature_factor,
                      1.0).reshape(HEADS).astype(np.float32)
    scale_g = ((DIM ** -0.5) / 1024.0 / temp).astype(np.float32)  # [16]
    scha_g = (184.6649652337873 * scale_g).astype(np.float32)

    def split8(a):
        a8 = np.asarray(a, F8)
        ar = np.asarray(a - a8.astype(np.float32), F8)
        return a8, ar

    def pack_k(a):  # [1024, C] -> [128, 4, 2, C]
        return np.ascontiguousarray(
            a.reshape(4, 2, P, a.shape[1]).transpose(2, 0, 1, 3))

    w32 = (32.0 * w_qkv).astype(np.float32)
    w8f, wrf = split8(w32)
    wo32 = (32.0 * w_out).astype(np.float32)
    wo8f, worf = split8(wo32)
    vk = (32.0 * void_k.reshape(HEADS, D)).astype(np.float32)
    vv = (32.0 * void_v.reshape(HEADS, D)).astype(np.float32)

    xt8, xtr = [], []
    for b in range(B):
        xt = np.ascontiguousarray(x[b].T).astype(np.float32)
        a8, ar = split8(xt)
        xt8.append(pack_k(a8))
        xtr.append(pack_k(ar))

    in_maps = []
    for core in range(8):
        b, hg = divmod(core, 4)
        h0 = hg * 4
        cs = slice(h0 * D, h0 * D + 256)
        cols = np.concatenate([
            np.arange(h0 * D, h0 * D + 256),
            np.arange(DIM + h0 * D, DIM + h0 * D + 256),
            np.arange(2 * DIM + h0 * D, 2 * DIM + h0 * D + 256)])
        vvo = np.ones((1, 4, P), np.float32)
        vvo[0, :, 0:D] = vv[h0:h0 + 4]
        in_maps.append({
            "xT8": xt8[b], "xr8": xtr[b],
            "wq8": pack_k(w8f[:, cols]), "wr8": pack_k(wrf[:, cols]),
            "wo8": np.ascontiguousarray(
                wo8f[cs].reshape(2, P, DIM).transpose(1, 0, 2)),
            "wor8": np.ascontiguousarray(
                worf[cs].reshape(2, P, DIM).transpose(1, 0, 2)),
            "vk8": np.ascontiguousarray(
                vk[h0:h0 + 4].reshape(2, P, 1)).astype(F8),
            "vvo": vvo.astype(BF),
            "scal": np.tile(scale_g[h0:h0 + 4], (P, 1)),
            "scha": np.tile(scha_g[h0:h0 + 4], (P, 1)),
        })
    return in_maps


def _run(in_maps, trace=False):
    from concourse import bass_utils
    if "nc" not in _cache:
        _cache["nc"] = _build()
    return bass_utils.run_bass_kernel_spmd(
        _cache["nc"], in_maps, core_ids=list(range(8)), trace=trace)


def kernel(x, w_qkv, w_out, b_out, void_q, void_k, void_v,
           attention_trace, temperature_factor):
    args = [np.asarray(a, dtype=np.float32) for a in
            (x, w_qkv, w_out, b_out, void_q, void_k, void_v,
             attention_trace, temperature_factor)]
    in_maps = _prep_inputs(*args)
    res = _run(in_maps)
    out = np.zeros((B, N, DIM), np.float32)
    for core in range(8):
        b = core // 4
        out[b] += np.asarray(res.results[core]["y"], dtype=np.float32)
    out *= (1.0 / 1024.0)
    out += args[3][None, None, :]                      # b_out
    return out
